# revision 5
# baseline (speedup 1.0000x reference)
"""2-layer GCN (PyG GCNConv semantics) as a Bass/Tile kernel for TRN2.

Math (per GCNConv layer, self-loops added, deg from dst in-degree + 1):
  out[d] = b + sum_{e: dst[e]=d} w[e] * t[src[e]]      with w[e] = rsqrt(deg[src]*deg[dst])
  where t = x        (layer 1: aggregate first, then @W1 — W commutes with aggregation)
        t = y1 @ W2  (layer 2: transform first)
  self-loop appears as an ordinary edge (i,i) with w = 1/deg[i].

Device mapping per core (nodes chunked across cores, edges bucketed by dst chunk):
  - dma_gather (SWDGE) gathers 512B feature rows by int16 index; the node table
    is split into "quarters" of 32768 rows to fit int16 indexing.
  - scatter-add via one-hot matmul: S[e, d] = w[e] * (dst_local[e] == d), built
    with one scalar_tensor_tensor per 128-edge slot; psum[f, d] += Msg^T @ S.
  - edges are pre-bucketed by (dst-tile, quarter) on host, each bucket padded to
    a multiple of 128 with dummy edges (idx 0, dst sentinel 255, degprod 1).
  - one NEFF runs on all cores: bucket sizes are the max over cores.
  - layer-2 input t is exchanged with an AllGather over internal DRAM.
"""

import math
import sys

import numpy as np

sys.path.insert(0, "/opt/trn_rl_repo")

import concourse.bass as bass
import concourse.bacc as bacc
import concourse.mybir as mybir
import concourse.tile as tile
from concourse.masks import make_identity

F32 = mybir.dt.float32
I16 = mybir.dt.int16
I32 = mybir.dt.int32

P = 128
QS = 32768  # int16-indexable rows per gather table slice
NEG_SLOPE = 0.01


# ---------------------------------------------------------------- host prep

class Meta:
    pass


def prep(edge_index, n_nodes, n_cores, tiles_per_super=2):
    """Bucket edges per (core, dst-tile, quarter); build common structure.

    Returns (meta, per_core_arrays) where per_core_arrays is a list of dicts
    with keys gidx (int16 [128, X]), dstloc (f32 [128, S]), wdeg (f32 [128, S]).
    """
    src = np.asarray(edge_index[0], dtype=np.int64)
    dst = np.asarray(edge_index[1], dtype=np.int64)
    deg = np.bincount(dst, minlength=n_nodes) + 1
    s_all = src
    d_all = dst
    degprod = (deg[s_all] * deg[d_all]).astype(np.float32)

    assert n_nodes % n_cores == 0
    chunk = n_nodes // n_cores
    NT = math.ceil(chunk / P)
    NQ = math.ceil(n_nodes / QS)

    TPS = tiles_per_super
    NS = math.ceil(NT / TPS)
    DW = TPS * P
    core_of = d_all // chunk
    sup_of = (d_all % chunk) // DW
    q_of = s_all // QS

    counts = np.zeros((n_cores, NS, NQ), dtype=np.int64)
    np.add.at(counts, (core_of, sup_of, q_of), 1)
    mx = counts.max(axis=0)  # [NS, NQ]
    slots_sq = ((mx + P - 1) // P).astype(np.int64)

    # stream order: for super: for q
    off = 0
    sp_meta = []
    for sp in range(NS):
        t0 = sp * TPS
        tiles = [
            (t, min(P, chunk - t * P))
            for t in range(t0, min(t0 + TPS, NT))
        ]
        groups = []  # (q, slot0, nslots)
        for q in range(NQ):
            s = int(slots_sq[sp, q])
            if s == 0:
                continue
            groups.append((q, off, s))
            off += s
        sp_meta.append(dict(sp=sp, tiles=tiles, col0=t0 * P, groups=groups))
    total_slots = off
    X = total_slots * P // 16

    # per-core data arrays
    per_core = []
    order = np.lexsort((s_all, q_of, sup_of, core_of))
    s_s = s_all[order]
    d_s = d_all[order]
    p_s = degprod[order]
    keys = ((core_of * NS + sup_of) * NQ + q_of)[order]
    bucket_starts = np.searchsorted(keys, np.arange(n_cores * NS * NQ), side="left")
    bucket_ends = np.searchsorted(keys, np.arange(n_cores * NS * NQ), side="right")

    for k in range(n_cores):
        gflat = np.zeros(total_slots * P, dtype=np.int16)  # idx (pad 0)
        dflat = np.full(total_slots * P, 999.0, dtype=np.float32)  # sentinel (> any DW)
        wflat = np.ones(total_slots * P, dtype=np.float32)  # degprod pad 1
        for spm in sp_meta:
            sp = spm["sp"]
            for (q, g0, s) in spm["groups"]:
                b = (k * NS + sp) * NQ + q
                i0, i1 = bucket_starts[b], bucket_ends[b]
                n = i1 - i0
                if n == 0:
                    continue
                pos = g0 * P
                gflat[pos : pos + n] = (s_s[i0:i1] - q * QS).astype(np.int16)
                dflat[pos : pos + n] = (d_s[i0:i1] % chunk - spm["col0"]).astype(np.float32)
                wflat[pos : pos + n] = p_s[i0:i1]
        # gidx layout: stream pos g -> [g % 16, g // 16], tiled 8x over partitions
        gidx = np.tile(gflat.reshape(-1, 16).T, (8, 1))  # [128, X]
        dstloc = dflat.reshape(-1, P).T.copy()  # [128, total_slots]
        wdeg = wflat.reshape(-1, P).T.copy()
        dn = np.ones(NT * P, dtype=np.float32)
        dn[:chunk] = deg[k * chunk : (k + 1) * chunk]
        degn = dn.reshape(NT, P).T.copy()  # [128, NT]
        per_core.append(dict(gidx=gidx, dstloc=dstloc, wdeg=wdeg, degn=degn))

    m = Meta()
    m.n_nodes = n_nodes
    m.n_cores = n_cores
    m.chunk = chunk
    m.NT = NT
    m.NQ = NQ
    m.supers = sp_meta
    m.total_slots = total_slots
    m.X = X
    m.dwidth = DW
    m.qbounds = [(q * QS, min(n_nodes, (q + 1) * QS)) for q in range(NQ)]
    return m, per_core


# ---------------------------------------------------------------- kernel build

F_IN, H1, H2, N_CLS = 128, 180, 120, 16


def build(m: Meta):
    nc = bacc.Bacc(trn_type="TRN2", num_devices=m.n_cores, target_bir_lowering=False)
    chunk, NT, NQ, DW = m.chunk, m.NT, m.NQ, m.dwidth

    x_d = nc.dram_tensor("x", [m.n_nodes, F_IN], F32, kind="ExternalInput")
    w1_d = nc.dram_tensor("W1", [F_IN, H1], F32, kind="ExternalInput")
    b1_d = nc.dram_tensor("b1", [H1, 1], F32, kind="ExternalInput")
    w2_d = nc.dram_tensor("W2", [H1, H2], F32, kind="ExternalInput")
    b2_d = nc.dram_tensor("b2", [H2, 1], F32, kind="ExternalInput")
    wl_d = nc.dram_tensor("Wl", [H2, N_CLS], F32, kind="ExternalInput")
    bl_d = nc.dram_tensor("bl", [1, N_CLS], F32, kind="ExternalInput")
    gidx_d = nc.dram_tensor("gidx", [P, m.X], I16, kind="ExternalInput")
    dstloc_d = nc.dram_tensor("dstloc", [P, m.total_slots], F32, kind="ExternalInput")
    wdeg_d = nc.dram_tensor("wdeg", [P, m.total_slots], F32, kind="ExternalInput")
    degn_d = nc.dram_tensor("degn", [P, m.NT], F32, kind="ExternalInput")
    xown_d = nc.dram_tensor("xown", [chunk, P], F32, kind="ExternalInput")
    out_d = nc.dram_tensor("out", [chunk, N_CLS], F32, kind="ExternalOutput")

    BF16 = mybir.dt.bfloat16
    tchunk_d = nc.dram_tensor("tchunk", [chunk, P], BF16, kind="Internal")
    tfull_d = nc.dram_tensor(
        "tfull", [m.n_nodes, P], BF16, kind="Internal", addr_space="Shared"
    )

    from contextlib import ExitStack

    with tile.TileContext(nc) as tc, ExitStack() as ctx:
        cpool = ctx.enter_context(tc.tile_pool(name="consts", bufs=1))
        mpool = ctx.enter_context(tc.tile_pool(name="msg", bufs=6))
        spool = ctx.enter_context(tc.tile_pool(name="onehot", bufs=12))
        wkpool = ctx.enter_context(tc.tile_pool(name="work", bufs=3))
        scat_pp = ctx.enter_context(tc.tile_pool(name="scat", bufs=2, space="PSUM"))
        y1_pp = ctx.enter_context(tc.tile_pool(name="y1ps", bufs=2, space="PSUM"))
        t_pp = ctx.enter_context(tc.tile_pool(name="tps", bufs=2, space="PSUM"))
        log_pp = ctx.enter_context(tc.tile_pool(name="logps", bufs=2, space="PSUM"))

        # ---- constants / resident tiles
        w1_s = cpool.tile([F_IN, H1], F32)
        w2a_s = cpool.tile([P, H2], F32)
        w2b_s = cpool.tile([H1 - P, H2], F32)
        wl_s = cpool.tile([H2, N_CLS], F32)
        bl_s = cpool.tile([1, N_CLS], F32)
        b1a_s = cpool.tile([P, 1], F32)
        b1b_s = cpool.tile([H1 - P, 1], F32)
        b2_s = cpool.tile([H2, 1], F32)
        gidx_s = cpool.tile([P, m.X], I16)
        dstloc_s = cpool.tile([P, m.total_slots], F32)
        wdeg_s = cpool.tile([P, m.total_slots], F32)
        w_s = cpool.tile([P, m.total_slots], F32)
        iota_i = cpool.tile([P, DW], I32)
        iota_f = cpool.tile([P, DW], F32)
        ident_f = cpool.tile([P, P], F32)
        identw_f = []
        identw_b = []
        for i in range(DW // P):
            iwf = cpool.tile([P, DW], F32, tag=f"iwf{i}")
            iwb = cpool.tile([P, DW], BF16, tag=f"iwb{i}")
            identw_f.append(iwf)
            identw_b.append(iwb)
        degn_s = cpool.tile([P, m.NT], F32)
        dinvn_s = cpool.tile([P, m.NT], F32)
        ones_s = cpool.tile([1, P], F32)

        nc.sync.dma_start(w1_s[:], w1_d[:])
        nc.sync.dma_start(w2a_s[:], w2_d[0:P, :])
        nc.sync.dma_start(w2b_s[:], w2_d[P:H1, :])
        nc.sync.dma_start(wl_s[:], wl_d[:])
        nc.sync.dma_start(bl_s[:], bl_d[:])
        nc.sync.dma_start(b1a_s[:], b1_d[0:P, :])
        nc.sync.dma_start(b1b_s[:], b1_d[P:H1, :])
        nc.sync.dma_start(b2_s[:], b2_d[:])
        nc.sync.dma_start(gidx_s[:], gidx_d[:])
        nc.sync.dma_start(dstloc_s[:], dstloc_d[:])
        nc.sync.dma_start(wdeg_s[:], wdeg_d[:])

        nc.sync.dma_start(degn_s[:], degn_d[:])
        nc.vector.reciprocal(dinvn_s[:], degn_s[:])
        make_identity(nc, ident_f[:])
        for ti in range(DW // P):
            nc.vector.memset(identw_f[ti][:], 0)
            nc.vector.tensor_copy(identw_f[ti][:, ti * P : (ti + 1) * P], ident_f[:])
            nc.vector.tensor_copy(identw_b[ti][:], identw_f[ti][:])
        nc.gpsimd.iota(iota_i[:], [[1, DW]], channel_multiplier=0)
        nc.vector.tensor_copy(iota_f[:], iota_i[:])
        nc.vector.memset(ones_s[:], 1.0)
        # w = sqrt(1/degprod)
        nc.vector.reciprocal(w_s[:], wdeg_s[:])
        nc.scalar.sqrt(w_s[:], w_s[:])

        def layer(table_aps, feat, epilogue, mdt=F32, own_d=None):
            """table_aps: per-quarter DRAM APs to gather from (rows x 128).
            feat: number of valid feature columns in gathered rows.
            epilogue(t, acc_region): consume [feat, 128] psum region for tile t.
            """
            identw = identw_f if mdt is F32 else identw_b
            for spm in m.supers:
                ngroups = len(spm["groups"])
                last_g = (
                    spm["groups"][-1][1] + spm["groups"][-1][2] - 1
                    if ngroups else -1
                )
                scat = scat_pp.tile([P, DW], F32, tag="scat")
                for ti, (t, rows) in enumerate(spm["tiles"]):
                    xt = wkpool.tile([P, P], mdt, tag="xt")
                    nc.sync.dma_start(
                        xt[:rows, :], own_d[t * P : t * P + rows, :]
                    )
                    dwt = spool.tile([P, DW], mdt, tag="S")
                    nc.scalar.activation(
                        dwt[:], identw[ti][:],
                        mybir.ActivationFunctionType.Copy,
                        scale=dinvn_s[:, t : t + 1],
                    )
                    nc.tensor.matmul(
                        out=scat[:feat, :],
                        lhsT=xt[:rows, :feat],
                        rhs=dwt[:rows, :],
                        start=(ti == 0),
                        stop=(ngroups == 0 and ti == len(spm["tiles"]) - 1),
                    )
                for (q, slot0, nsl) in spm["groups"]:
                    n_idx = nsl * P
                    msg = mpool.tile([P, nsl, P], mdt, tag="msg")
                    nc.gpsimd.dma_gather(
                        out_ap=msg[:],
                        in_ap=table_aps[q],
                        idxs_ap=gidx_s[:, slot0 * 8 : slot0 * 8 + n_idx // 16],
                        num_idxs=n_idx,
                        num_idxs_reg=n_idx,
                        elem_size=P,
                        single_packet=(n_idx <= 1024),
                    )
                    for si in range(nsl):
                        g = slot0 + si
                        S = spool.tile([P, DW], mdt, tag="S")
                        nc.vector.scalar_tensor_tensor(
                            out=S[:],
                            in0=iota_f[:],
                            scalar=dstloc_s[:, g : g + 1],
                            in1=w_s[:, g : g + 1].to_broadcast([P, DW]),
                            op0=mybir.AluOpType.is_equal,
                            op1=mybir.AluOpType.mult,
                        )
                        nc.tensor.matmul(
                            out=scat[:feat, :],
                            lhsT=msg[:, si, :feat],
                            rhs=S[:],
                            start=False,
                            stop=(g == last_g),
                        )
                for ti, (t, rows) in enumerate(spm["tiles"]):
                    epilogue(t, scat[:feat, ti * P : (ti + 1) * P])

        def l1_epilogue(t, acc):
            rows = min(P, chunk - t * P)
            h1pre = wkpool.tile([P, P], F32, tag="h1pre")
            nc.scalar.copy(h1pre[:], acc)
            y1ps = y1_pp.tile([P, 2 * P], F32, tag="y1ps")
            nc.tensor.matmul(
                out=y1ps[:, 0:P], lhsT=w1_s[:, 0:P], rhs=h1pre[:],
                start=True, stop=True,
            )
            nc.tensor.matmul(
                out=y1ps[: H1 - P, P : 2 * P], lhsT=w1_s[:, P:H1], rhs=h1pre[:],
                start=True, stop=True,
            )
            y1a_u = wkpool.tile([P, P], F32, tag="y1a_u")
            y1b_u = wkpool.tile([H1 - P, P], F32, tag="y1b_u")
            nc.scalar.activation(
                y1a_u[:], y1ps[:, 0:P], mybir.ActivationFunctionType.Identity,
                bias=b1a_s[:, 0:1], scale=1.0,
            )
            nc.scalar.activation(
                y1b_u[:], y1ps[: H1 - P, P : 2 * P],
                mybir.ActivationFunctionType.Identity,
                bias=b1b_s[:, 0:1], scale=1.0,
            )
            y1a = wkpool.tile([P, P], F32, tag="y1a")
            y1b = wkpool.tile([H1 - P, P], F32, tag="y1b")
            nc.vector.scalar_tensor_tensor(
                out=y1a[:], in0=y1a_u[:], scalar=NEG_SLOPE, in1=y1a_u[:],
                op0=mybir.AluOpType.mult, op1=mybir.AluOpType.max,
            )
            nc.vector.scalar_tensor_tensor(
                out=y1b[:], in0=y1b_u[:], scalar=NEG_SLOPE, in1=y1b_u[:],
                op0=mybir.AluOpType.mult, op1=mybir.AluOpType.max,
            )
            tps = t_pp.tile([P, H2], F32, tag="tps")
            nc.tensor.matmul(out=tps[:], lhsT=y1a[:], rhs=w2a_s[:], start=True, stop=False)
            nc.tensor.matmul(out=tps[:], lhsT=y1b[:], rhs=w2b_s[:], start=False, stop=True)
            t_sb = wkpool.tile([P, P], BF16, tag="t_sb")
            nc.scalar.copy(t_sb[:, 0:H2], tps[:])
            nc.vector.memset(t_sb[:, H2:P], 0)
            nc.sync.dma_start(
                tchunk_d[t * P : t * P + rows, :], t_sb[:rows, :]
            )

        def l2_epilogue(t, acc):
            rows = min(P, chunk - t * P)
            y2_u = wkpool.tile([H2, P], F32, tag="y2_u")
            nc.scalar.activation(
                y2_u[:], acc, mybir.ActivationFunctionType.Identity,
                bias=b2_s[:, 0:1], scale=1.0,
            )
            y2 = wkpool.tile([H2, P], F32, tag="y2")
            nc.vector.scalar_tensor_tensor(
                out=y2[:], in0=y2_u[:], scalar=NEG_SLOPE, in1=y2_u[:],
                op0=mybir.AluOpType.mult, op1=mybir.AluOpType.max,
            )
            lg = log_pp.tile([P, N_CLS], F32, tag="lg")
            nc.tensor.matmul(out=lg[:], lhsT=y2[:], rhs=wl_s[:], start=True, stop=False)
            nc.tensor.matmul(out=lg[:], lhsT=ones_s[:], rhs=bl_s[:], start=False, stop=True)
            negm = wkpool.tile([P, 1], F32, tag="negm")
            nc.vector.tensor_reduce(
                negm[:], lg[:], mybir.AxisListType.X, mybir.AluOpType.max, negate=True
            )
            ex = wkpool.tile([P, N_CLS], F32, tag="ex")
            nc.scalar.activation(
                ex[:], lg[:], mybir.ActivationFunctionType.Exp,
                bias=negm[:, 0:1], scale=1.0,
            )
            ssum = wkpool.tile([P, 1], F32, tag="ssum")
            nc.vector.tensor_reduce(
                ssum[:], ex[:], mybir.AxisListType.X, mybir.AluOpType.add
            )
            lns = wkpool.tile([P, 1], F32, tag="lns")
            nc.scalar.activation(
                lns[:], ssum[:], mybir.ActivationFunctionType.Ln
            )
            negtot = wkpool.tile([P, 1], F32, tag="negtot")
            nc.vector.tensor_sub(negtot[:], negm[:], lns[:])
            osb = wkpool.tile([P, N_CLS], F32, tag="osb")
            nc.scalar.activation(
                osb[:], lg[:], mybir.ActivationFunctionType.Identity,
                bias=negtot[:, 0:1], scale=1.0,
            )
            nc.sync.dma_start(out_d[t * P : t * P + rows, :], osb[:rows, :])

        # ---- layer 1: aggregate raw x, then transform to t = y1 @ W2
        with nc.named_scope("layer1"):
            x_q = [x_d[lo:hi, :] for (lo, hi) in m.qbounds]
            layer(x_q, F_IN, l1_epilogue, own_d=xown_d)

        # ---- exchange t
        with nc.named_scope("exchange"):
            if m.n_cores > 1:
                nc.gpsimd.collective_compute(
                    "AllGather",
                    mybir.AluOpType.bypass,
                    replica_groups=[list(range(m.n_cores))],
                    ins=[tchunk_d[:]],
                    outs=[tfull_d[:]],
                )
            else:
                nc.sync.dma_start(tfull_d[:], tchunk_d[:])

        # ---- layer 2: aggregate t, epilogue produces logits + log_softmax
        with nc.named_scope("layer2"):
            t_q = [tfull_d[lo:hi, :] for (lo, hi) in m.qbounds]
            layer(t_q, H2, l2_epilogue, mdt=BF16, own_d=tchunk_d)

    nc.compile()  # bacc passes (register allocation etc.)
    return nc


# ---------------------------------------------------------------- numpy oracle

def ref_numpy(x, W1, b1, W2, b2, Wl, bl, edge_index):
    src, dst = edge_index[0], edge_index[1]
    n = x.shape[0]

    def gcn(xx, W, b):
        h = xx @ W
        deg = (np.bincount(dst, minlength=n) + 1).astype(np.float64)
        dinv = 1.0 / np.sqrt(deg)
        out = np.zeros((n, W.shape[1]), dtype=np.float64)
        norm = dinv[src] * dinv[dst]
        np.add.at(out, dst, h[src] * norm[:, None])
        out += h * (dinv * dinv)[:, None]
        return out + b

    def lrelu(v):
        return np.where(v > 0, v, NEG_SLOPE * v)

    h = lrelu(gcn(x.astype(np.float64), W1, b1))
    h = lrelu(gcn(h, W2, b2))
    logits = h @ Wl + bl
    mx = logits.max(axis=1, keepdims=True)
    lse = np.log(np.exp(logits - mx).sum(axis=1, keepdims=True)) + mx
    return (logits - lse).astype(np.float32)


# ---------------------------------------------------------------- entry point

N_NODES = 100000
N_EDGES = 800000
N_CORES = 8

TRACE = False
LAST_EXEC_NS = None
LAST_RESULTS = None


def kernel(x, W1, b1, W2, b2, Wl, bl, edge_index):
    """Full-input GCN kernel: shards across 8 NeuronCores internally."""
    global LAST_EXEC_NS, LAST_RESULTS
    from concourse import bass_utils

    x = np.ascontiguousarray(np.asarray(x, dtype=np.float32))
    W1 = np.asarray(W1, dtype=np.float32)
    b1 = np.asarray(b1, dtype=np.float32).reshape(-1, 1)
    W2 = np.asarray(W2, dtype=np.float32)
    b2 = np.asarray(b2, dtype=np.float32).reshape(-1, 1)
    Wl = np.asarray(Wl, dtype=np.float32)
    bl = np.asarray(bl, dtype=np.float32).reshape(1, -1)
    edge_index = np.asarray(edge_index)

    n_nodes = x.shape[0]
    meta, per_core = prep(edge_index, n_nodes, n_cores=N_CORES)
    nc = build(meta)

    chunk = n_nodes // N_CORES
    shared = dict(x=x, W1=W1, b1=b1, W2=W2, b2=b2, Wl=Wl, bl=bl)
    in_maps = [
        {**shared, "gidx": pc["gidx"], "dstloc": pc["dstloc"],
         "wdeg": pc["wdeg"], "degn": pc["degn"],
         "xown": x[k * chunk : (k + 1) * chunk]}
        for k, pc in enumerate(per_core)
    ]
    res = bass_utils.run_bass_kernel_spmd(
        nc, in_maps, core_ids=list(range(N_CORES)), trace=TRACE
    )
    LAST_EXEC_NS = res.exec_time_ns
    LAST_RESULTS = res
    return np.concatenate([r["out"] for r in res.results], axis=0)



# revision 13
# speedup vs baseline: 1.0336x; 1.0336x over previous
"""2-layer GCN (PyG GCNConv semantics) as a Bass/Tile kernel for TRN2.

Math (per GCNConv layer, self-loops added, deg from dst in-degree + 1):
  out[d] = b + sum_{e: dst[e]=d} w[e] * t[src[e]]      with w[e] = rsqrt(deg[src]*deg[dst])
  where t = x        (layer 1: aggregate first, then @W1 — W commutes with aggregation)
        t = y1 @ W2  (layer 2: transform first)
  self-loop appears as an ordinary edge (i,i) with w = 1/deg[i].

Device mapping per core (nodes chunked across cores, edges bucketed by dst super):
  - supers of DW=512 dst columns (4 tiles of 128); edges bucketed per
    (super, quarter) where quarter = 32768-row slice of the gather table
    (int16 index limit), each bucket padded to a multiple of 128.
  - dma_gather (SWDGE) fetches 256B bf16 feature rows; calls are grouped
    over GSUP supers x 1 quarter to amortize the ~1.5us/call fixed cost
    (the Q7 descriptor-gen rate of ~7ns/row is the kernel's floor).
  - scatter-add via one-hot matmul: S[e, d] = w[e] * (dst_local[e] == d),
    one scalar_tensor_tensor per 128-edge slot; psum[f, d] += Msg^T @ S.
  - self-loops via identity matmul of dinv-scaled own rows.
  - everything bf16 in the matmuls (1 cyc/row); psum accumulates f32.
  - log_softmax is batched over all tiles at the end (2 ACT table loads).
  - layer-2 input t is exchanged with an AllGather over internal DRAM.
"""

import math
import sys

import numpy as np

sys.path.insert(0, "/opt/trn_rl_repo")

import concourse.bass as bass
import concourse.bacc as bacc
import concourse.mybir as mybir
import concourse.tile as tile
from concourse.masks import make_identity

F32 = mybir.dt.float32
BF16 = mybir.dt.bfloat16
I16 = mybir.dt.int16
I32 = mybir.dt.int32

P = 128
QS = 32768  # int16-indexable rows per gather table slice
NEG_SLOPE = 0.01
TPS = 4     # dst tiles per super (DW = TPS*128)
GSUP = 3    # supers per grouped gather call

F_IN, H1, H2, N_CLS = 128, 180, 120, 16


# ---------------------------------------------------------------- host prep

class Meta:
    pass


def prep(edge_index, n_nodes, n_cores):
    """Bucket edges per (core, super, quarter); group gathers over GSUP supers.

    Stream order of slots: for each gather-group gg: for each quarter q:
    for each super sp in gg: that (sp, q) bucket's slots (padded to x128).
    """
    src = np.asarray(edge_index[0], dtype=np.int64)
    dst = np.asarray(edge_index[1], dtype=np.int64)
    deg = np.bincount(dst, minlength=n_nodes) + 1
    w_e = (1.0 / np.sqrt(deg[src] * deg[dst])).astype(np.float32)

    assert n_nodes % n_cores == 0
    chunk = n_nodes // n_cores
    NT = math.ceil(chunk / P)
    NQ = math.ceil(n_nodes / QS)
    DW = TPS * P
    NS = math.ceil(NT / TPS)
    NG = math.ceil(NS / GSUP)

    core_of = dst // chunk
    sup_of = (dst % chunk) // DW
    q_of = src // QS

    counts = np.zeros((n_cores, NS, NQ), dtype=np.int64)
    np.add.at(counts, (core_of, sup_of, q_of), 1)
    mx = counts.max(axis=0)  # [NS, NQ]
    slots_sq = ((mx + P - 1) // P).astype(np.int64)

    # layout slots in stream order: gg -> q -> sp
    slot0 = np.zeros((NS, NQ), dtype=np.int64)
    ggroups = []
    off = 0
    for g in range(NG):
        sps = list(range(g * GSUP, min((g + 1) * GSUP, NS)))
        qg = []
        for q in range(NQ):
            g0 = off
            for sp in sps:
                slot0[sp, q] = off
                off += int(slots_sq[sp, q])
            if off > g0:
                qg.append((q, g0, off - g0))
        ggroups.append(dict(sps=sps, qgroups=qg))
    total_slots = off

    supers = []
    for sp in range(NS):
        t0 = sp * TPS
        tiles = [(t, min(P, chunk - t * P)) for t in range(t0, min(t0 + TPS, NT))]
        groups = [
            (q, int(slot0[sp, q]), int(slots_sq[sp, q]))
            for q in range(NQ)
            if slots_sq[sp, q] > 0
        ]
        supers.append(dict(sp=sp, tiles=tiles, col0=t0 * P, groups=groups))

    # per-core data arrays
    order = np.lexsort((src, q_of, sup_of, core_of))
    s_s = src[order]
    d_s = dst[order]
    w_s = w_e[order]
    keys = ((core_of * NS + sup_of) * NQ + q_of)[order]
    bucket_lo = np.searchsorted(keys, np.arange(n_cores * NS * NQ), side="left")
    bucket_hi = np.searchsorted(keys, np.arange(n_cores * NS * NQ), side="right")

    per_core = []
    for k in range(n_cores):
        gflat = np.zeros(total_slots * P, dtype=np.int16)
        dflat = np.full(total_slots * P, 999.0, dtype=np.float32)
        wflat = np.zeros(total_slots * P, dtype=np.float32)
        for sp in range(NS):
            for (q, g0, nsl) in supers[sp]["groups"]:
                b = (k * NS + sp) * NQ + q
                i0, i1 = bucket_lo[b], bucket_hi[b]
                n = i1 - i0
                if n == 0:
                    continue
                pos = g0 * P
                gflat[pos : pos + n] = (s_s[i0:i1] - q * QS).astype(np.int16)
                dflat[pos : pos + n] = (
                    d_s[i0:i1] % chunk - supers[sp]["col0"]
                ).astype(np.float32)
                wflat[pos : pos + n] = w_s[i0:i1]
        # gidx layout: stream pos g -> [g % 16, g // 16], tiled 8x over partitions
        gidx = np.tile(gflat.reshape(-1, 16).T, (8, 1))  # [128, total_slots*8]
        dstloc = dflat.reshape(-1, P).T.copy()  # [128, total_slots]
        wv = wflat.reshape(-1, P).T.copy()
        dn = np.ones(NT * P, dtype=np.float32)
        dn[:chunk] = deg[k * chunk : (k + 1) * chunk]
        degn = dn.reshape(NT, P).T.copy()  # [128, NT]
        per_core.append(dict(gidx=gidx, dstloc=dstloc, wv=wv, degn=degn))

    m = Meta()
    m.n_nodes = n_nodes
    m.n_cores = n_cores
    m.chunk = chunk
    m.NT = NT
    m.NQ = NQ
    m.NS = NS
    m.dwidth = DW
    m.supers = supers
    m.ggroups = ggroups
    m.total_slots = total_slots
    m.qbounds = [(q * QS, min(n_nodes, (q + 1) * QS)) for q in range(NQ)]
    return m, per_core


# ---------------------------------------------------------------- kernel build

def build(m: Meta):
    nc = bacc.Bacc(trn_type="TRN2", num_devices=m.n_cores, target_bir_lowering=False)
    chunk, NT, NQ, NS, DW = m.chunk, m.NT, m.NQ, m.NS, m.dwidth

    xb_d = nc.dram_tensor("xb", [m.n_nodes, P], BF16, kind="ExternalInput")
    xown_d = nc.dram_tensor("xown", [chunk, P], BF16, kind="ExternalInput")
    w1_d = nc.dram_tensor("w1b", [F_IN, H1], BF16, kind="ExternalInput")
    w2a_d = nc.dram_tensor("w2ab", [P, H2], BF16, kind="ExternalInput")
    w2b_d = nc.dram_tensor("w2bb", [H1 - P, H2], BF16, kind="ExternalInput")
    wl_d = nc.dram_tensor("wlb", [H2, N_CLS], BF16, kind="ExternalInput")
    blx_d = nc.dram_tensor("blx4b", [1, TPS * N_CLS], BF16, kind="ExternalInput")
    b1a_d = nc.dram_tensor("b1a", [P, 1], F32, kind="ExternalInput")
    b1b_d = nc.dram_tensor("b1b", [H1 - P, 1], F32, kind="ExternalInput")
    b2_d = nc.dram_tensor("b2", [H2, 1], F32, kind="ExternalInput")
    gidx_d = nc.dram_tensor("gidx", [P, m.total_slots * 8], I16, kind="ExternalInput")
    dstloc_d = nc.dram_tensor("dstloc", [P, m.total_slots], F32, kind="ExternalInput")
    wv_d = nc.dram_tensor("wv", [P, m.total_slots], F32, kind="ExternalInput")
    degn_d = nc.dram_tensor("degn", [P, m.NT], F32, kind="ExternalInput")
    out_d = nc.dram_tensor("out", [chunk, N_CLS], F32, kind="ExternalOutput")

    tchunk_d = nc.dram_tensor("tchunk", [chunk, P], BF16, kind="Internal")
    tfull_d = nc.dram_tensor(
        "tfull", [m.n_nodes, P], BF16, kind="Internal", addr_space="Shared"
    )

    from contextlib import ExitStack

    with tile.TileContext(nc) as tc, ExitStack() as ctx:
        cpool = ctx.enter_context(tc.tile_pool(name="consts", bufs=1))
        mpool = ctx.enter_context(tc.tile_pool(name="msg", bufs=2))
        spool = ctx.enter_context(tc.tile_pool(name="onehot", bufs=8))
        wkpool = ctx.enter_context(tc.tile_pool(name="work", bufs=3))
        scat_pp = ctx.enter_context(tc.tile_pool(name="scat", bufs=2, space="PSUM"))
        y1a_pp = ctx.enter_context(tc.tile_pool(name="y1aps", bufs=2, space="PSUM"))
        y1b_pp = ctx.enter_context(tc.tile_pool(name="y1bps", bufs=1, space="PSUM"))
        t_pp = ctx.enter_context(tc.tile_pool(name="tps", bufs=1, space="PSUM"))
        log_pp = ctx.enter_context(tc.tile_pool(name="logps", bufs=1, space="PSUM"))

        # ---- constants / resident tiles
        w1_s = cpool.tile([F_IN, H1], BF16)
        w2a_s = cpool.tile([P, H2], BF16)
        w2b_s = cpool.tile([H1 - P, H2], BF16)
        wl_s = cpool.tile([H2, N_CLS], BF16)
        blx_s = cpool.tile([1, TPS * N_CLS], BF16)
        ones_s = cpool.tile([1, P], BF16)
        b1a_s = cpool.tile([P, 1], F32)
        b1b_s = cpool.tile([H1 - P, 1], F32)
        b2_s = cpool.tile([H2, 1], F32)
        gidx_s = cpool.tile([P, m.total_slots * 8], I16)
        dstloc_s = cpool.tile([P, m.total_slots], F32)
        w_s = cpool.tile([P, m.total_slots], F32)
        iota_i = cpool.tile([P, DW], I32)
        iota_f = cpool.tile([P, DW], F32)
        ident_f = cpool.tile([P, P], F32)
        identw_f = cpool.tile([P, TPS, DW], F32)
        identw_b = cpool.tile([P, TPS, DW], BF16)
        degn_s = cpool.tile([P, m.NT], F32)
        dinvn_s = cpool.tile([P, m.NT], F32)
        lgall_s = cpool.tile([P, NT, N_CLS], F32)

        nc.sync.dma_start(w1_s[:], w1_d[:])
        nc.sync.dma_start(w2a_s[:], w2a_d[:])
        nc.sync.dma_start(w2b_s[:], w2b_d[:])
        nc.sync.dma_start(wl_s[:], wl_d[:])
        nc.sync.dma_start(blx_s[:], blx_d[:])
        nc.sync.dma_start(b1a_s[:], b1a_d[:])
        nc.sync.dma_start(b1b_s[:], b1b_d[:])
        nc.sync.dma_start(b2_s[:], b2_d[:])
        nc.sync.dma_start(gidx_s[:], gidx_d[:])
        nc.sync.dma_start(dstloc_s[:], dstloc_d[:])
        wv_s = w_s
        nc.sync.dma_start(wv_s[:], wv_d[:])
        nc.sync.dma_start(degn_s[:], degn_d[:])

        nc.vector.reciprocal(dinvn_s[:], degn_s[:])
        make_identity(nc, ident_f[:])
        nc.vector.memset(identw_f[:], 0)
        for ti in range(TPS):
            nc.vector.tensor_copy(identw_f[:, ti, ti * P : (ti + 1) * P], ident_f[:])
        nc.vector.tensor_copy(identw_b[:], identw_f[:])
        nc.gpsimd.iota(iota_i[:], [[1, DW]], channel_multiplier=0)
        nc.vector.tensor_copy(iota_f[:], iota_i[:])
        nc.vector.memset(ones_s[:], 1.0)

        Prelu = mybir.ActivationFunctionType.Prelu
        Copy = mybir.ActivationFunctionType.Copy

        def layer(table_aps, feat, epilogue, own_d):
            for gg in m.ggroups:
                msgs = {}
                for (q, g0, gn) in gg["qgroups"]:
                    mt = mpool.tile([P, gn, P], BF16, tag=f"msg{q}")
                    nc.gpsimd.dma_gather(
                        out_ap=mt[:],
                        in_ap=table_aps[q],
                        idxs_ap=gidx_s[:, g0 * 8 : (g0 + gn) * 8],
                        num_idxs=gn * P,
                        num_idxs_reg=gn * P,
                        elem_size=P,
                        single_packet=(gn * P <= 1024),
                    )
                    msgs[q] = (mt, g0)
                for sp in gg["sps"]:
                    spm = m.supers[sp]
                    ntl = len(spm["tiles"])
                    ncols = ntl * P
                    scat = scat_pp.tile([P, DW], F32, tag="scat")
                    # self-loops: psum[f, ti*128+p] += dinv[p] * own[t*128+p, f]
                    xt4 = wkpool.tile([P, TPS, P], BF16, tag="xt4")
                    r_full = [rows for (_, rows) in spm["tiles"] if rows == P]
                    nfull = len(r_full)
                    t0 = spm["tiles"][0][0]
                    if nfull:
                        nc.sync.dma_start(
                            xt4[:, 0:nfull, :],
                            own_d[t0 * P : (t0 + nfull) * P, :].rearrange(
                                "(a b) c -> b a c", b=P
                            ),
                        )
                    if nfull < ntl:  # partial last tile
                        lt, lrows = spm["tiles"][nfull]
                        nc.sync.dma_start(
                            xt4[:lrows, nfull, :], own_d[lt * P : lt * P + lrows, :]
                        )
                    for ti, (t, rows) in enumerate(spm["tiles"]):
                        dwt = spool.tile([P, DW], BF16, tag="S")
                        nc.scalar.activation(
                            dwt[:, :ncols], identw_b[:, ti, :ncols], Copy,
                            scale=dinvn_s[:, t : t + 1],
                        )
                        nc.tensor.matmul(
                            out=scat[:feat, :ncols],
                            lhsT=xt4[:rows, ti, :feat],
                            rhs=dwt[:rows, :ncols],
                            start=(ti == 0),
                            stop=False,
                        )
                    last = spm["groups"][-1]
                    for (q, g0, nsl) in spm["groups"]:
                        mt, mg0 = msgs[q]
                        for si in range(nsl):
                            g = g0 + si
                            S = spool.tile([P, DW], BF16, tag="S")
                            nc.vector.scalar_tensor_tensor(
                                out=S[:, :ncols],
                                in0=iota_f[:, :ncols],
                                scalar=dstloc_s[:, g : g + 1],
                                in1=wv_s[:, g : g + 1].to_broadcast([P, ncols]),
                                op0=mybir.AluOpType.is_equal,
                                op1=mybir.AluOpType.mult,
                            )
                            nc.tensor.matmul(
                                out=scat[:feat, :ncols],
                                lhsT=mt[:, g - mg0, :feat],
                                rhs=S[:, :ncols],
                                start=False,
                                stop=(q == last[0] and si == nsl - 1),
                            )
                    epilogue(spm, scat)

        def l1_epilogue(spm, scat):
            ntl = len(spm["tiles"])
            ncols = ntl * P
            h1b = wkpool.tile([P, DW], BF16, tag="h1b")
            nc.scalar.activation(h1b[:, :ncols], scat[:, :ncols], Copy)
            y1aps = y1a_pp.tile([P, DW], F32, tag="y1aps")
            y1bps = y1b_pp.tile([H1 - P, DW], F32, tag="y1bps")
            nc.tensor.matmul(
                out=y1aps[:, :ncols], lhsT=w1_s[:, 0:P], rhs=h1b[:, :ncols],
                start=True, stop=True,
            )
            nc.tensor.matmul(
                out=y1bps[:, :ncols], lhsT=w1_s[:, P:H1], rhs=h1b[:, :ncols],
                start=True, stop=True,
            )
            y1ab = wkpool.tile([P, DW], BF16, tag="y1ab")
            y1bb = wkpool.tile([H1 - P, DW], BF16, tag="y1bb")
            nc.scalar.activation(
                y1ab[:, :ncols], y1aps[:, :ncols], Prelu,
                bias=b1a_s[:, 0:1], scale=1.0, alpha=NEG_SLOPE,
            )
            nc.scalar.activation(
                y1bb[:, :ncols], y1bps[:, :ncols], Prelu,
                bias=b1b_s[:, 0:1], scale=1.0, alpha=NEG_SLOPE,
            )
            tps = t_pp.tile([P, TPS, H2], F32, tag="tps")
            for ti, (t, rows) in enumerate(spm["tiles"]):
                nc.tensor.matmul(
                    out=tps[:, ti, :], lhsT=y1ab[:, ti * P : (ti + 1) * P],
                    rhs=w2a_s[:], start=True, stop=False,
                )
                nc.tensor.matmul(
                    out=tps[:, ti, :], lhsT=y1bb[:, ti * P : (ti + 1) * P],
                    rhs=w2b_s[:], start=False, stop=True,
                )
            t_sb = wkpool.tile([P, TPS, P], BF16, tag="t_sb")
            nc.scalar.activation(t_sb[:, 0:ntl, 0:H2], tps[:, 0:ntl, :], Copy)
            t0 = spm["tiles"][0][0]
            r_full = [rows for (_, rows) in spm["tiles"] if rows == P]
            nfull = len(r_full)
            if nfull:
                nc.sync.dma_start(
                    tchunk_d[t0 * P : (t0 + nfull) * P, :].rearrange(
                        "(a b) c -> b a c", b=P
                    ),
                    t_sb[:, 0:nfull, :],
                )
            if nfull < ntl:
                lt, lrows = spm["tiles"][nfull]
                nc.sync.dma_start(
                    tchunk_d[lt * P : lt * P + lrows, :], t_sb[:lrows, nfull, :]
                )

        def l2_epilogue(spm, scat):
            ntl = len(spm["tiles"])
            ncols = ntl * P
            y2b = wkpool.tile([H2, DW], BF16, tag="y2b")
            nc.scalar.activation(
                y2b[:, :ncols], scat[:H2, :ncols], Prelu,
                bias=b2_s[:, 0:1], scale=1.0, alpha=NEG_SLOPE,
            )
            lg = log_pp.tile([P, TPS, N_CLS], F32, tag="lg")
            for ti, (t, rows) in enumerate(spm["tiles"]):
                nc.tensor.matmul(
                    out=lg[:, ti, :], lhsT=y2b[:, ti * P : (ti + 1) * P],
                    rhs=wl_s[:], start=True, stop=False,
                )
                nc.tensor.matmul(
                    out=lg[:, ti, :], lhsT=ones_s[:],
                    rhs=blx_s[:, 0:N_CLS], start=False, stop=True,
                )
            t0 = spm["tiles"][0][0]
            nc.scalar.activation(lgall_s[:, t0 : t0 + ntl, :], lg[:, 0:ntl, :], Copy)

        # ---- layer 1: aggregate raw x, then transform to t = y1 @ W2
        with nc.named_scope("layer1"):
            x_q = [xb_d[lo:hi, :] for (lo, hi) in m.qbounds]
            layer(x_q, F_IN, l1_epilogue, own_d=xown_d)

        # ---- exchange t
        with nc.named_scope("exchange"):
            if m.n_cores > 1:
                nc.gpsimd.collective_compute(
                    "AllGather",
                    mybir.AluOpType.bypass,
                    replica_groups=[list(range(m.n_cores))],
                    ins=[tchunk_d[:]],
                    outs=[tfull_d[:]],
                )
            else:
                nc.sync.dma_start(tfull_d[:], tchunk_d[:])

        # ---- layer 2: aggregate t, batched log_softmax at the end
        with nc.named_scope("layer2"):
            t_q = [tfull_d[lo:hi, :] for (lo, hi) in m.qbounds]
            layer(t_q, H2, l2_epilogue, own_d=tchunk_d)

            Exp = mybir.ActivationFunctionType.Exp
            Ln = mybir.ActivationFunctionType.Ln
            negm = wkpool.tile([P, NT, 1], F32, tag="negm")
            nc.vector.tensor_reduce(
                negm[:], lgall_s[:], mybir.AxisListType.X,
                mybir.AluOpType.max, negate=True,
            )
            xm = wkpool.tile([P, NT, N_CLS], F32, tag="xm")
            nc.vector.tensor_tensor(
                out=xm[:], in0=lgall_s[:],
                in1=negm[:].to_broadcast([P, NT, N_CLS]),
                op=mybir.AluOpType.add,
            )
            ex = wkpool.tile([P, NT, N_CLS], F32, tag="ex")
            nc.scalar.activation(ex[:], xm[:], Exp)
            ssum = wkpool.tile([P, NT, 1], F32, tag="ssum")
            nc.vector.tensor_reduce(
                ssum[:], ex[:], mybir.AxisListType.X, mybir.AluOpType.add
            )
            lns = wkpool.tile([P, NT, 1], F32, tag="lns")
            nc.scalar.activation(lns[:], ssum[:], Ln)
            osb = wkpool.tile([P, NT, N_CLS], F32, tag="osb")
            nc.vector.tensor_tensor(
                out=osb[:], in0=xm[:],
                in1=lns[:].to_broadcast([P, NT, N_CLS]),
                op=mybir.AluOpType.subtract,
            )
            # out rows t*128+p for t<97 full; tile 97 has 84 rows
            nfull_t = chunk // P
            nc.sync.dma_start(
                out_d[0 : nfull_t * P, :].rearrange("(a b) c -> b a c", b=P),
                osb[:, 0:nfull_t, :],
            )
            rem = chunk - nfull_t * P
            if rem:
                nc.sync.dma_start(
                    out_d[nfull_t * P : chunk, :], osb[:rem, nfull_t, :]
                )

    nc.compile()
    return nc


# ---------------------------------------------------------------- entry point

N_NODES = 100000
N_EDGES = 800000
N_CORES = 8

TRACE = False
LAST_EXEC_NS = None
LAST_RESULTS = None


def kernel(x, W1, b1, W2, b2, Wl, bl, edge_index):
    """Full-input GCN kernel: shards across 8 NeuronCores internally."""
    global LAST_EXEC_NS, LAST_RESULTS
    import ml_dtypes
    from concourse import bass_utils

    bf = ml_dtypes.bfloat16
    x = np.ascontiguousarray(np.asarray(x, dtype=np.float32))
    W1 = np.asarray(W1, dtype=np.float32)
    b1 = np.asarray(b1, dtype=np.float32).reshape(-1, 1)
    W2 = np.asarray(W2, dtype=np.float32)
    b2 = np.asarray(b2, dtype=np.float32).reshape(-1, 1)
    Wl = np.asarray(Wl, dtype=np.float32)
    bl = np.asarray(bl, dtype=np.float32).reshape(1, -1)
    edge_index = np.asarray(edge_index)

    n_nodes = x.shape[0]
    meta, per_core = prep(edge_index, n_nodes, n_cores=N_CORES)
    nc = build(meta)

    chunk = n_nodes // N_CORES
    xb = x.astype(bf)
    shared = dict(
        xb=xb,
        w1b=W1.astype(bf),
        w2ab=W2[:P].astype(bf),
        w2bb=W2[P:].astype(bf),
        wlb=Wl.astype(bf),
        blx4b=np.tile(bl.astype(bf), (1, TPS)),
        b1a=b1[:P],
        b1b=b1[P:],
        b2=b2,
    )
    in_maps = [
        {**shared, "gidx": pc["gidx"], "dstloc": pc["dstloc"],
         "wv": pc["wv"], "degn": pc["degn"],
         "xown": xb[k * chunk : (k + 1) * chunk]}
        for k, pc in enumerate(per_core)
    ]
    res = bass_utils.run_bass_kernel_spmd(
        nc, in_maps, core_ids=list(range(N_CORES)), trace=TRACE
    )
    LAST_EXEC_NS = res.exec_time_ns
    LAST_RESULTS = res
    return np.concatenate([r["out"] for r in res.results], axis=0)
